# revision 45
# baseline (speedup 1.0000x reference)
"""Trainium2 Bass kernel for nn_ConvKernelBank.

Computation:
  alpha = softmax_M( causal_conv1d( gelu(pre_w @ mean_hw(q) + pre_b), mix_w ) + mix_b )
  k_out = sum_m alpha[b,m,t] * dwconv3d_causal(k, Wk[m])
  v_out = sum_m alpha[b,m,t] * dwconv3d_causal(v, Wv[m])

Strategy (v2):
  - 8 NeuronCores, data-parallel over (batch, T-half): core i handles
    b = i // 2, t in [16*(i%2), 16*(i%2)+16).  Causal temporal halo of 2
    frames is passed in from the host (zeros at sequence start).
  - Layout [C=128 partitions, (t, h, w) free]; k/v arrive spatially
    zero-padded to 26x26 per frame (pad supplies the conv zero-padding).
  - alpha is folded into per-output-frame effective 27-tap depthwise
    filters weff[c, t*54 + tap] (k taps 0..26, v taps 27..53).
  - The 32 frame-tensors (16 t x {k,v}) are split:
      * PE frame-tensors: 2 halves x 27 accumulating diagonal matmuls
        (diag = per-channel tap weights) in fp16, drained by ACT/DVE to
        fp16 staging, DMA'd out.
      * Vector frame-tensors: 27 per-tap products (tensor_scalar, which
        runs in the DVE 4x perf mode for fp16) produced on DVE/Pool/ACT,
        summed by a chain of fp16 DVE tensor_tensor adds (2x mode).
  - Diagonal weight matrices are built from an identity matrix scaled by
    weff columns on ACT (activation scale), DVE (tensor_scalar 4x) and
    Pool (broadcast tensor_tensor).
  - alpha partition-broadcast via an SBUF->SBUF partition-collapsing DMA
    ([16,3] -> [1,48]) followed by a K=1 ones-outer-product matmul.
  - Outputs are written fp16 (host upcasts to fp32).
  - PE is kept warm through the serial head with dummy matmuls so the
    conv matmuls run at the full 2.4 GHz p-state.
"""

import os
from contextlib import ExitStack

import numpy as np

import concourse.bass as bass
import concourse.tile as tile
from concourse import mybir
from concourse.bass_utils import run_bass_kernel_spmd
from concourse.vector_clock import ScopedClock

B, C, T, H, W = 4, 128, 32, 24, 24
M, KT, KS, MIXK = 3, 3, 3, 3
HW = H * W
NCORES = 8
TLOC = 16          # output frames per core
THALO = 2          # causal temporal halo
TIN = TLOC + THALO
F32 = mybir.dt.float32
F16 = mybir.dt.float16
NTAP = KT * KS * KS  # 27
WBLK = 2 * NTAP      # 54: 27 k-taps then 27 v-taps per frame
PADS = KS - 1        # spatial padding (1 on each side)
PH, PW = H + PADS, W + PADS          # 26, 26
PHW = PH * PW                        # 676
HHALF = H // 2                       # 12 rows per PSUM half
FHALF = HHALF * W                    # 288 free elements per half

AluOp = mybir.AluOpType
ActFn = mybir.ActivationFunctionType

# ---- schedule knobs ---------------------------------------------------------
# frames (t, xi) handled by the vector path (products + adds); rest go to PE
VEC_FT = {(t, xi) for t in (2, 5, 8, 11, 14) for xi in (0, 1)}
# per-PE-frame diag builds: engine for each of the 27 taps
DIAG_ACT, DIAG_POOL, DIAG_DVE = 13, 14, 0
# per-vector-frame products: engine for each of the 27 taps
PROD_DVE, PROD_POOL, PROD_ACT = 23, 0, 4
# drains (2 per PE frame): every DRAIN_DVE_EVERYth drain goes to DVE
DRAIN_DVE_EVERY = 1000000
DIAG_LOOKAHEAD = 3
PE_WARMUP = 32     # dummy matmuls to hold the PE p-state through the head
# -----------------------------------------------------------------------------

# consts blob layout (f32): wkv | prew | preb | mixw | hmask
OFF_WKV = 0
OFF_PREW = OFF_WKV + M * WBLK          # 162
OFF_PREB = OFF_PREW + C                # 290
OFF_MIXW = OFF_PREB + 1                # 291
OFF_HMASK = OFF_MIXW + MIXK * M        # 300
BLOB32 = OFF_HMASK + THALO             # 302


class _SplitDrainTileContext(tile.TileContext):
    """TileContext whose final drain splits semaphore waits across several
    drain instructions: this walrus build rejects >2 sync waits on one
    CTRL instruction ("Too many sync wait commands")."""

    MAX_WAITS = 1

    def _drain_and_barrier(self, tick_clock, wait_clock):
        nc = self.nc
        drain_inst = nc.sync.drain()
        wait_clock.add_sem_waits(
            drain_inst.ins, ScopedClock({None: tick_clock.global_clock})
        )
        mi = drain_inst.ins
        si = mi.sync_info
        waits = list(si.on_wait or []) if si is not None else []
        if len(waits) > self.MAX_WAITS:
            si.on_wait = waits[: self.MAX_WAITS]
            rest = waits[self.MAX_WAITS :]
            for i in range(0, len(rest), self.MAX_WAITS):
                d2 = nc.sync.drain()
                d2.ins.sync_info = mybir.SyncInfo(
                    on_wait=rest[i : i + self.MAX_WAITS], on_update=[]
                )
        nc.all_engine_barrier()
        popped = nc._tile_sem_poison_stack.pop()
        assert popped is self._sem_poison
        nc.clear_and_free_semaphores(list(self.sems.allocated().values()))
        nc.all_engine_barrier()


_MAX_SYNC_WAITS = 1

_NOP_ENGINES = {
    mybir.EngineType.PE,
    mybir.EngineType.DVE,
    mybir.EngineType.Activation,
    mybir.EngineType.Pool,
    mybir.EngineType.SP,
}


def _split_sync_waits(nc: bass.Bass, max_waits: int = _MAX_SYNC_WAITS) -> None:
    """Walrus rejects instructions carrying more than ~2 semaphore waits.
    Move excess waits onto freshly inserted same-engine NoOps placed just
    before the offending instruction (waiting earlier is always safe)."""
    for fn in nc.m.functions:
        for bb in fn.blocks:
            new_list = []
            changed = False
            for inst in bb.instructions:
                si = inst.sync_info
                waits = list(si.on_wait) if (si is not None and si.on_wait) else []
                if len(waits) > max_waits:
                    assert inst.engine in _NOP_ENGINES, (
                        f"can't split waits on {inst.engine} {type(inst).__name__}"
                    )
                    excess, keep = waits[:-max_waits], waits[-max_waits:]
                    for i in range(0, len(excess), max_waits):
                        nop = mybir.InstNoOp(
                            name=nc.get_next_instruction_name(), ins=[], outs=[]
                        )
                        nop.engine = inst.engine
                        nop.sync_info = mybir.SyncInfo(
                            on_wait=excess[i : i + max_waits], on_update=[]
                        )
                        new_list.append(nop)
                    si.on_wait = keep
                    changed = True
                new_list.append(inst)
            if changed:
                bb.instructions[:] = new_list


def _build_program() -> bass.Bass:
    nc = bass.Bass()

    qin = nc.declare_dram_parameter("qin", [C, TIN * HW], F16, isOutput=False)
    kin = nc.declare_dram_parameter("kin", [C, TIN * PHW], F16, isOutput=False)
    vin = nc.declare_dram_parameter("vin", [C, TIN * PHW], F16, isOutput=False)
    blob32 = nc.declare_dram_parameter("blob32", [C, BLOB32], F32, isOutput=False)
    ident = nc.declare_dram_parameter("ident", [C, C], F16, isOutput=False)
    mixb = nc.declare_dram_parameter("mixb", [TLOC, M + TLOC * M], F32, isOutput=False)

    kout = nc.declare_dram_parameter("kout", [C, TLOC * HW], F16, isOutput=True)
    vout = nc.declare_dram_parameter("vout", [C, TLOC * HW], F16, isOutput=True)

    with ExitStack() as ctx:
        tc = ctx.enter_context(_SplitDrainTileContext(nc))

        consts = ctx.enter_context(tc.tile_pool(name="consts", bufs=1))
        big = ctx.enter_context(tc.tile_pool(name="big", bufs=1))
        small = ctx.enter_context(tc.tile_pool(name="small", bufs=1))
        diagp = ctx.enter_context(tc.tile_pool(name="diagp", bufs=110))
        prodp = ctx.enter_context(tc.tile_pool(name="prodp", bufs=2))
        accp = ctx.enter_context(tc.tile_pool(name="accp", bufs=3))
        stg = ctx.enter_context(tc.tile_pool(name="stg", bufs=8))
        apsum = ctx.enter_context(tc.tile_pool(name="apsum", bufs=1, space="PSUM"))
        cpsum = ctx.enter_context(tc.tile_pool(name="cpsum", bufs=7, space="PSUM"))

        # ---- consts, then q (the mix predictor is the serial head) ----
        QCH = 2   # q frames per chunk
        NQCH = TIN // QCH
        q_ch = [big.tile([C, QCH * HW], F16, name=f"q{i}") for i in range(NQCH)]
        XCH = 6   # k/v frames per chunk
        k_ch = [big.tile([C, XCH * PHW], F16, name=f"k{i}") for i in range(3)]
        v_ch = [big.tile([C, XCH * PHW], F16, name=f"v{i}") for i in range(3)]
        id_sb = consts.tile([C, C], F16)
        blob_sb = consts.tile([C, BLOB32], F32)
        mixb_full = consts.tile([TLOC, M + TLOC * M], F32)
        nc.sync.dma_start(id_sb[:], ident[:])
        nc.sync.dma_start(blob_sb[:], blob32[:])
        nc.sync.dma_start(mixb_full[:], mixb[:])
        for i in range(NQCH):
            nc.sync.dma_start(q_ch[i][:], qin[:, i * QCH * HW : (i + 1) * QCH * HW])
        HXCH = XCH // 2
        with tc.tile_wait_until(0.0072):
            nc.sync.dma_start(k_ch[0][:, : HXCH * PHW], kin[:, : HXCH * PHW])
        with tc.tile_wait_until(0.0082):
            nc.sync.dma_start(v_ch[0][:, : HXCH * PHW], vin[:, : HXCH * PHW])
        with tc.tile_wait_until(0.0105):
            nc.sync.dma_start(
                k_ch[0][:, HXCH * PHW : XCH * PHW], kin[:, HXCH * PHW : XCH * PHW]
            )
            nc.sync.dma_start(
                v_ch[0][:, HXCH * PHW : XCH * PHW], vin[:, HXCH * PHW : XCH * PHW]
            )
        with tc.tile_wait_until(0.016):
            nc.scalar.dma_start(k_ch[1][:], kin[:, XCH * PHW : 2 * XCH * PHW])
            nc.scalar.dma_start(v_ch[1][:], vin[:, XCH * PHW : 2 * XCH * PHW])
        with tc.tile_wait_until(0.028):
            nc.scalar.dma_start(k_ch[2][:], kin[:, 2 * XCH * PHW :])
            nc.scalar.dma_start(v_ch[2][:], vin[:, 2 * XCH * PHW :])
        mixb_sb = mixb_full[:, 0:M]
        selmask = mixb_full[:, M : M + TLOC * M]

        wkv_sb = blob_sb[:, OFF_WKV : OFF_WKV + M * WBLK]
        prew_sb = blob_sb[:, OFF_PREW : OFF_PREW + C]
        preb_sb = blob_sb[:, OFF_PREB : OFF_PREB + 1]
        mixw_sb = blob_sb[:, OFF_MIXW : OFF_MIXW + MIXK * M]
        hmask_sb = blob_sb[:, OFF_HMASK : OFF_HMASK + THALO]

        # ---- PE warmup: hold the p-state ramp through the serial head ----
        warm_ps = apsum.tile([C, C], F32, padded_shape=[C, 512], tag="aps")
        for i in range(PE_WARMUP):
            nc.tensor.matmul(
                warm_ps[:], id_sb[:], id_sb[:],
                start=(i == 0), stop=(i == PE_WARMUP - 1),
            )

        # ---- mix predictor ----
        # qg[c, t] = sum_hw q (1/HW folded into prew on host); DVE 4x products
        qg = small.tile([C, TIN], F32)
        qscratch = small.tile([C, HW], F16)
        qscratch2 = small.tile([C, HW], F32)
        for t in range(TIN):
            src = q_ch[t // QCH][:, (t % QCH) * HW : (t % QCH + 1) * HW]
            if t % 3 == 2:
                nc.scalar.activation(
                    qscratch2[:], src, ActFn.Copy, accum_out=qg[:, t : t + 1]
                )
            else:
                nc.vector.tensor_scalar(
                    qscratch[:], src, 1.0, 0.0, AluOp.mult, AluOp.add,
                    accum_out=qg[:, t : t + 1],
                )
        # h = gelu(prew.T @ qg + preb)
        h_ps = apsum.tile([C, TIN], F32, padded_shape=[C, 512], tag="aps")
        nc.tensor.matmul(h_ps[:], prew_sb, qg[:], start=True, stop=True)
        h_sb = small.tile([C, TIN], F32)
        nc.scalar.activation(
            h_sb[:], h_ps[:], ActFn.Gelu, bias=preb_sb[:, 0:1], scale=1.0
        )
        # zero the causal halo columns where the reference zero-pads h
        nc.vector.tensor_mul(h_sb[:, 0:THALO], h_sb[:, 0:THALO], hmask_sb)

        # logits[t, m] = sum_j sum_c mix_w[m,c,j] h[c, t+j]  (t local)
        lg_ps = apsum.tile([TLOC, M], F32, tag="aps")
        for j in range(MIXK):
            nc.tensor.matmul(
                lg_ps[:],
                h_sb[:, j : j + TLOC],
                mixw_sb[:, j * M : (j + 1) * M],
                start=(j == 0),
                stop=(j == MIXK - 1),
            )
        lt = small.tile([TLOC, M], F32)
        nc.vector.tensor_add(lt[:], lg_ps[:], mixb_sb)

        # softmax over m (free dim)
        rmax = small.tile([TLOC, 1], F32)
        nc.vector.tensor_reduce(rmax[:], lt[:], axis=mybir.AxisListType.X, op=AluOp.max)
        nmax = small.tile([TLOC, 1], F32)
        nc.vector.tensor_scalar(nmax[:], rmax[:], -1.0, None, AluOp.mult)
        ex = small.tile([TLOC, M], F32)
        nc.scalar.activation(ex[:], lt[:], ActFn.Exp, bias=nmax[:, 0:1], scale=1.0)
        ssum = small.tile([TLOC, 1], F32)
        nc.vector.tensor_reduce(ssum[:], ex[:], axis=mybir.AxisListType.X, op=AluOp.add)
        rcp = small.tile([TLOC, 1], F32)
        nc.vector.reciprocal(rcp[:], ssum[:])
        alpha_t = small.tile([TLOC, M], F32)
        nc.vector.tensor_scalar(alpha_t[:], ex[:], rcp[:, 0:1], None, AluOp.mult)

        # broadcast alpha to all 128 partitions without DMA: diagonalize
        # alpha over t (stride-0 broadcast * selmask), then sum over the 16
        # partitions with a ones matmul: out[c,(t,m)] = sum_t' ad[t',(t,m)]
        ad = small.tile([TLOC, TLOC * M], F32)
        nc.vector.tensor_tensor(
            ad[:].rearrange("p (t m) -> p t m", t=TLOC),
            alpha_t[:].rearrange("p (a m) -> p a m", a=1).broadcast_to([TLOC, TLOC, M]),
            selmask.rearrange("p (t m) -> p t m", t=TLOC),
            AluOp.mult,
        )
        ones = small.tile([TLOC, C], F32)
        nc.vector.memset(ones[:], 1.0)
        abc_ps = apsum.tile([C, TLOC * M], F32, tag="aps")
        nc.tensor.matmul(abc_ps[:], ones[:], ad[:], start=True, stop=True)
        abc = small.tile([C, TLOC * M], F32)
        nc.vector.tensor_copy(abc[:], abc_ps[:])
        warm2_ps = apsum.tile([C, C], F32, padded_shape=[C, 512], tag="aps")
        for i in range(10):
            nc.tensor.matmul(
                warm2_ps[:], id_sb[:], id_sb[:],
                start=(i == 0), stop=(i == 9),
            )

        # ---- fold alpha into per-frame effective filters ----
        # weff[c, t*54 + tap] = sum_m alpha[t, m] * wkv[c, m*54 + tap]
        weff = big.tile([C, TLOC * WBLK], F32)
        for t in range(TLOC):
            dst = weff[:, t * WBLK : (t + 1) * WBLK]
            for m in range(M):
                a_sc = abc[:, t * M + m : t * M + m + 1]
                src = wkv_sb[:, m * WBLK : (m + 1) * WBLK]
                if m == 0:
                    nc.vector.tensor_scalar(dst, src, a_sc, None, AluOp.mult)
                else:
                    nc.vector.scalar_tensor_tensor(
                        dst, src, a_sc, dst, AluOp.mult, AluOp.add
                    )

        # ---- the depthwise conv ----
        def xframe(xi, i):
            """Padded input frame i of tensor xi as [C, PH, PW] (fp16)."""
            ch = (k_ch, v_ch)[xi][i // XCH]
            return ch[:].rearrange("p (t h w) -> p t h w", t=XCH, h=PH)[:, i % XCH]

        def wcol(t, base, tap):
            c0 = t * WBLK + base + tap
            return weff[:, c0 : c0 + 1]

        # tap order: (dt, dh, dw) with tap = dt*9 + dh*3 + dw
        def tap_src(xi, t, tap):
            dt, r = divmod(tap, 9)
            dh, dw = divmod(r, 3)
            xf = xframe(xi, t + dt)
            return xf[:, dh : dh + H, dw : dw + W]

        # ---- vector frame-tensor: products + fp16 add chain ----
        # product engines per tap index (interleave so DVE consumes early)
        _prod_engines = (
            ["dve"] * PROD_DVE + ["act"] * PROD_ACT + ["pool"] * PROD_POOL
        )

        def _emit_prod(eng, dst3, src, w):
            if eng == "dve":
                nc.vector.tensor_scalar(dst3, src, w, None, AluOp.mult)
            elif eng == "act":
                nc.scalar.activation(dst3, src, ActFn.Copy, scale=w)
            else:
                nc.gpsimd.tensor_scalar(dst3, src, w, None, AluOp.mult)

        # taps 0..26 split into groups A(8) B(8) C(8) D(3); products land in
        # wide group tiles, summed by wide fp16 adds + a log2 fold of A.
        _GRP = (8, 8, 8, 3)

        def vec_frame(t, xi):
            odram = (kout, vout)[xi]
            base = xi * NTAP
            acc = accp.tile([C, 8 * HW], F16, tag="acc")
            grp_tiles = [acc]
            for gi, gn in enumerate(_GRP[1:], 1):
                gt = prodp.tile([C, 8 * HW], F16, tag=f"grp{gi}")
                grp_tiles.append(gt)
            tap = 0
            ei = 0
            peng = ["dve"] + list(_prod_engines)  # tap0 engine prepended
            for gi, gn in enumerate(_GRP):
                gt = grp_tiles[gi]
                for s in range(gn):
                    w = wcol(t, base, tap)
                    src = tap_src(xi, t, tap)
                    dst3 = gt[:, s * HW : (s + 1) * HW].rearrange(
                        "p (h w) -> p h w", h=H
                    )
                    _emit_prod(peng[tap], dst3, src, w)
                    tap += 1
            # wide merges: acc += B; acc += C; acc[0:3] += D
            nc.vector.tensor_tensor(acc[:], grp_tiles[1][:], acc[:], AluOp.add)
            nc.vector.tensor_tensor(acc[:], grp_tiles[2][:], acc[:], AluOp.add)
            nc.vector.tensor_tensor(
                acc[:, : 3 * HW], grp_tiles[3][:, : 3 * HW], acc[:, : 3 * HW],
                AluOp.add,
            )
            # log fold of the 8 slices
            for width in (4, 2, 1):
                nc.vector.tensor_tensor(
                    acc[:, : width * HW],
                    acc[:, width * HW : 2 * width * HW],
                    acc[:, : width * HW],
                    AluOp.add,
                )
            nc.sync.dma_start(odram[:, t * HW : (t + 1) * HW], acc[:, :HW])

        # ---- diag builds for one PE frame-tensor, spread over engines ----
        def build_diags(t, xi, three_way=False):
            base = xi * NTAP
            tiles = []
            n_act, n_dve = (9, 9) if three_way else (DIAG_ACT, DIAG_DVE)
            for tap in range(NTAP):
                dg = diagp.tile([C, C], F16, tag="dg")
                w = wcol(t, base, tap)
                if tap < n_act:
                    nc.scalar.activation(dg[:], id_sb[:], ActFn.Copy, scale=w)
                elif tap < n_act + n_dve:
                    nc.vector.tensor_scalar(dg[:], id_sb[:], w, None, AluOp.mult)
                else:
                    nc.gpsimd.tensor_tensor(
                        dg[:], id_sb[:], w.broadcast_to([C, C]), AluOp.mult
                    )
                tiles.append(dg)
            return tiles

        # ---- PE frame-tensor: 2 halves x 27 accumulating diag matmuls ----
        drain_ctr = [0]

        def pe_frame(t, xi, diags, split_dma=False):
            odram = (kout, vout)[xi]
            st = stg.tile([C, HW], F16, tag="st")
            for half in range(2):
                h0 = half * HHALF
                acc = cpsum.tile([C, FHALF], F32, tag="pe", padded_shape=[C, 512])
                for dt in range(KT):
                    xr = xframe(xi, t + dt)
                    for dh in range(KS):
                        for dw in range(KS):
                            tap = dt * 9 + dh * 3 + dw
                            rhs = xr[:, h0 + dh : h0 + dh + HHALF, dw : dw + W]
                            nc.tensor.matmul(
                                acc[:], diags[tap][:], rhs,
                                start=(tap == 0), stop=(tap == NTAP - 1),
                            )
                dst = st[:, h0 * W : h0 * W + FHALF]
                drain_ctr[0] += 1
                if drain_ctr[0] % DRAIN_DVE_EVERY == 0 or (
                    drain_ctr[0] >= 41 and drain_ctr[0] % 2 == 1
                ):
                    nc.vector.tensor_copy(dst, acc[:])
                else:
                    nc.scalar.activation(dst, acc[:], ActFn.Copy)
                if split_dma:
                    nc.scalar.dma_start(
                        odram[:, t * HW + h0 * W : t * HW + h0 * W + FHALF], dst
                    )
            if not split_dma:
                nc.scalar.dma_start(odram[:, t * HW : (t + 1) * HW], st[:])

        # ---- schedule ----
        order = [(t, xi) for t in range(TLOC) for xi in range(2)]
        pe_list = [ft for ft in order if ft not in VEC_FT]
        vec_list = [ft for ft in order if ft in VEC_FT]

        diag_store = {}
        for i in range(DIAG_LOOKAHEAD + 1):
            diag_store[pe_list[i]] = build_diags(*pe_list[i])

        vi = 0  # next vector frame to emit
        NPE, NV = len(pe_list), len(vec_list)
        for i, ft in enumerate(pe_list):
            # no vec work during PE ft 0 (head); then pace to finish 2 early
            while i >= 2 and vi < NV and (vi * (NPE - 4) <= (i - 2) * NV):
                vec_frame(*vec_list[vi])
                vi += 1
            ahead = i + DIAG_LOOKAHEAD + 1
            if ahead < len(pe_list):
                diag_store[pe_list[ahead]] = build_diags(*pe_list[ahead])
            pe_frame(*ft, diag_store.pop(ft), split_dma=(i >= len(pe_list) - 4))
        while vi < len(vec_list):
            vec_frame(*vec_list[vi])
            vi += 1

    _split_sync_waits(nc)
    return nc


_PROGRAM_CACHE: bass.Bass | None = None

# Results of the last hardware run (for the test harness to inspect).
LAST_RESULT = None


def _get_program() -> bass.Bass:
    global _PROGRAM_CACHE
    if _PROGRAM_CACHE is None:
        _PROGRAM_CACHE = _build_program()
    return _PROGRAM_CACHE


def _halo_pad_slice(x_b: np.ndarray, t0: int) -> np.ndarray:
    """x_b: [C, T, H, W] -> [C, TIN*PHW] fp16: 2 leading halo frames (zeros
    when t0 == 0) and each frame zero-padded spatially to 26x26."""
    out = np.zeros((C, TIN, PH, PW), dtype=np.float16)
    if t0 == 0:
        out[:, THALO:, 1 : 1 + H, 1 : 1 + W] = x_b[:, t0 : t0 + TLOC]
    else:
        out[:, :, 1 : 1 + H, 1 : 1 + W] = x_b[:, t0 - THALO : t0 + TLOC]
    return np.ascontiguousarray(out.reshape(C, TIN * PHW))


def _halo_slice(x_b: np.ndarray, t0: int) -> np.ndarray:
    if t0 == 0:
        halo = np.zeros((C, THALO, H, W), dtype=np.float16)
    else:
        halo = x_b[:, t0 - THALO : t0].astype(np.float16)
    out = np.concatenate([halo, x_b[:, t0 : t0 + TLOC].astype(np.float16)], axis=1)
    return np.ascontiguousarray(out.reshape(C, TIN * HW))


def _make_in_maps(q, k, v, Wk, Wv, pre_w, pre_b, mix_w, mix_b):
    q = np.asarray(q, dtype=np.float32)
    k = np.asarray(k, dtype=np.float32)
    v = np.asarray(v, dtype=np.float32)
    Wk = np.asarray(Wk, dtype=np.float32)
    Wv = np.asarray(Wv, dtype=np.float32)
    pre_w = np.asarray(pre_w, dtype=np.float32)
    pre_b = np.asarray(pre_b, dtype=np.float32)
    mix_w = np.asarray(mix_w, dtype=np.float32)
    mix_b = np.asarray(mix_b, dtype=np.float32)

    # shared (replicated) weight prep, packed into one f32 blob
    wk_flat = Wk.reshape(M, C, NTAP)  # [m, c, tap]
    wv_flat = Wv.reshape(M, C, NTAP)
    blob = np.zeros((C, BLOB32), dtype=np.float32)
    for m in range(M):
        blob[:, OFF_WKV + m * WBLK : OFF_WKV + m * WBLK + NTAP] = wk_flat[m]
        blob[:, OFF_WKV + m * WBLK + NTAP : OFF_WKV + (m + 1) * WBLK] = wv_flat[m]
    blob[:, OFF_PREW : OFF_PREW + C] = (pre_w / HW).T  # lhsT layout [c_in, c_out]
    blob[:, OFF_PREB] = pre_b
    for j in range(MIXK):
        for m in range(M):
            blob[:, OFF_MIXW + j * M + m] = mix_w[m, :, j]
    mixb_host = np.zeros((TLOC, M + TLOC * M), dtype=np.float32)
    mixb_host[:, 0:M] = mix_b[None, :]
    for t in range(TLOC):
        mixb_host[t, M + t * M : M + (t + 1) * M] = 1.0
    ident_host = np.eye(C, dtype=np.float16)

    in_maps = []
    for core in range(NCORES):
        b, th = core // 2, core % 2
        t0 = th * TLOC
        blob_c = blob.copy()
        blob_c[:, OFF_HMASK : OFF_HMASK + THALO] = 0.0 if t0 == 0 else 1.0
        in_maps.append(
            {
                "qin": _halo_slice(q[b], t0),
                "kin": _halo_pad_slice(k[b], t0),
                "vin": _halo_pad_slice(v[b], t0),
                "blob32": blob_c,
                "ident": ident_host,
                "mixb": mixb_host,
            }
        )
    return in_maps


def kernel(q, k, v, Wk, Wv, pre_w, pre_b, mix_w, mix_b):
    in_maps = _make_in_maps(q, k, v, Wk, Wv, pre_w, pre_b, mix_w, mix_b)
    nc = _get_program()
    trace = bool(int(os.environ.get("BASSK_TRACE", "0")))
    res = run_bass_kernel_spmd(nc, in_maps, list(range(NCORES)), trace=trace)
    global LAST_RESULT
    LAST_RESULT = res

    k_out = np.empty((B, C, T, H, W), dtype=np.float32)
    v_out = np.empty((B, C, T, H, W), dtype=np.float32)
    for core in range(NCORES):
        b, th = core // 2, core % 2
        t0 = th * TLOC
        k_out[b, :, t0 : t0 + TLOC] = (
            res.results[core]["kout"].astype(np.float32).reshape(C, TLOC, H, W)
        )
        v_out[b, :, t0 : t0 + TLOC] = (
            res.results[core]["vout"].astype(np.float32).reshape(C, TLOC, H, W)
        )
    return (k_out, v_out)
